# revision 1
# baseline (speedup 1.0000x reference)
"""Longformer layer stack (4 layers, sliding-window attention) on 8 TRN2 cores.

Sharding: data-parallel over batch (2) x sequence-parallel (4 blocks of 1024
tokens). Each core computes its sequence block; the banded attention needs a
W=256 token halo, exchanged between neighboring blocks with an AllGather after
each layer (layers 0-2). Residual stream kept transposed ([dmodel, tokens]) in
float32r; attention probs/values in bf16.
"""
import sys

sys.path.insert(0, '/opt/trn_rl_repo')

import numpy as np
import ml_dtypes

import concourse.bass as bass
import concourse.mybir as mybir
import concourse.tile as tile
from concourse import bacc
from concourse import bass_utils

F32 = mybir.dt.float32
F32R = mybir.dt.float32r
BF16 = mybir.dt.bfloat16
I32 = mybir.dt.int32
AF = mybir.ActivationFunctionType
ALU = mybir.AluOpType

NH = 12          # heads
DH = 64          # head dim
HD = 768         # model dim
FF = 3072        # ffn dim
W = 256          # one-sided window
L = 4            # layers
B = 2
S = 4096
EPS = 1e-12
N_CORES = 8
T_OWN = 1024     # tokens per core
T_EXT = 1536     # with halos
FT = 6           # model-dim 128-tiles
FFT = 24         # ffn-dim 128-tiles
NCH = 4          # local chunks of 256 queries
P = 128


def _ln_transposed(nc, sb_stats, ps_stats, r_aps, out_aps, ones_r, eps_sb,
                   s_ap, b_ap, ncols):
    """LayerNorm over the partition (feature) axis of transposed tiles.

    r_aps: list of FT fp32r SBUF APs [128, ncols] (input; normalized IN PLACE
    up to the final scale/bias which lands in out_aps).
    out_aps[ft] <- LN(r)*s + b (fp32r). s_ap/b_ap: [128, FT] sbuf.
    """
    sumx = ps_stats.tile([1, ncols], F32, tag="lnsum", name="lnsum")
    sumsq = ps_stats.tile([1, ncols], F32, tag="lnsumsq", name="lnsumsq")
    nseg = (ncols + 511) // 512
    for ft in range(FT):
        sq = sb_stats.tile([P, ncols], F32R, tag="lnsq", name="lnsq")
        nc.scalar.activation(sq[:], r_aps[ft], AF.Square)
        for j in range(nseg):
            cs = slice(j * 512, min((j + 1) * 512, ncols))
            nc.tensor.matmul(sumx[0:1, cs], ones_r[:, 0:1], r_aps[ft][:, cs],
                             start=(ft == 0), stop=(ft == FT - 1))
            nc.tensor.matmul(sumsq[0:1, cs], ones_r[:, 0:1], sq[:, cs],
                             start=(ft == 0), stop=(ft == FT - 1))
    mu = sb_stats.tile([1, ncols], F32, tag="lnmu", name="lnmu")
    nc.scalar.activation(mu[:], sumx[:], AF.Identity, scale=1.0 / HD)
    musq = sb_stats.tile([1, ncols], F32, tag="lnmusq", name="lnmusq")
    nc.vector.tensor_tensor(musq[:], mu[:], mu[:], op=ALU.mult)
    var = sb_stats.tile([1, ncols], F32, tag="lnvar", name="lnvar")
    nc.vector.scalar_tensor_tensor(out=var[:], in0=sumsq[:], scalar=1.0 / HD,
                                   in1=musq[:], op0=ALU.mult, op1=ALU.subtract)
    sd = sb_stats.tile([1, ncols], F32, tag="lnsd", name="lnsd")
    nc.scalar.activation(sd[:], var[:], AF.Sqrt, bias=eps_sb[0:1, :])
    rstd = sb_stats.tile([1, ncols], F32, tag="lnrstd", name="lnrstd")
    nc.vector.reciprocal(rstd[:], sd[:])
    mu_b = sb_stats.tile([P, ncols], F32, tag="lnmub", name="lnmub")
    nc.gpsimd.partition_broadcast(mu_b[:], mu[:], channels=P)
    rstd_b = sb_stats.tile([P, ncols], F32, tag="lnrstdb", name="lnrstdb")
    nc.gpsimd.partition_broadcast(rstd_b[:], rstd[:], channels=P)
    for ft in range(FT):
        nc.vector.tensor_tensor(r_aps[ft], r_aps[ft], mu_b[:], op=ALU.subtract)
        nc.vector.tensor_tensor(r_aps[ft], r_aps[ft], rstd_b[:], op=ALU.mult)
        nc.scalar.activation(out_aps[ft], r_aps[ft], AF.Identity,
                             scale=s_ap[:, ft:ft + 1], bias=b_ap[:, ft:ft + 1])


def build_nc(n_layers=L):
    nc = bacc.Bacc("TRN2", target_bir_lowering=False, debug=False,
                   num_devices=N_CORES)
    dt_ = nc.dram_tensor
    t = {}
    t["emb"] = dt_("emb_word", [32000, HD], F32, kind="ExternalInput").ap()
    t["ids"] = dt_("ids", [P, 12], I32, kind="ExternalInput").ap()
    t["pos"] = dt_("pos", [T_EXT, HD], F32, kind="ExternalInput").ap()
    t["eln_s"] = dt_("eln_s", [HD], F32, kind="ExternalInput").ap()
    t["eln_b"] = dt_("eln_b", [HD], F32, kind="ExternalInput").ap()
    t["wq"] = dt_("wq", [L, FT, P, FT, P], F32R, kind="ExternalInput").ap()
    t["wk"] = dt_("wk", [L, FT, P, FT, P], F32R, kind="ExternalInput").ap()
    t["wv"] = dt_("wv", [L, HD, HD], F32R, kind="ExternalInput").ap()
    t["wo"] = dt_("wo", [L, FT, P, FT, P], BF16, kind="ExternalInput").ap()
    t["w1"] = dt_("w1", [L, FFT, P, FT, P], F32R, kind="ExternalInput").ap()
    t["w2"] = dt_("w2", [L, FF, HD], F32R, kind="ExternalInput").ap()
    for nm in ["bq", "bk", "bo", "b2", "ls1", "lb1", "ls2", "lb2"]:
        t[nm] = dt_(nm, [L, P, FT], F32, kind="ExternalInput").ap()
    t["b1"] = dt_("b1", [L, P, FFT], F32, kind="ExternalInput").ap()
    t["ml"] = dt_("ml", [NCH, P, 512], BF16, kind="ExternalInput").ap()
    t["mr"] = dt_("mr", [NCH, P, 512], BF16, kind="ExternalInput").ap()
    t["halo_ids"] = dt_("halo_ids", [P, 12], I32, kind="ExternalInput").ap()
    t["out"] = dt_("out", [FT, P, T_OWN], F32, kind="ExternalOutput").ap()

    with tile.TileContext(nc) as tc:
        _build_body(nc, tc, n_layers, t)
    nc.compile()
    return nc


def _build_body(nc, tc, n_layers, t):
    from contextlib import ExitStack
    with ExitStack() as ctx:
        persist = ctx.enter_context(tc.tile_pool(name="persist", bufs=1))
        # residual stream, transposed, with halos: x[ft] = [128, T_EXT]
        x = [persist.tile([P, T_EXT], F32R, tag=f"x{ft}", name=f"x{ft}") for ft in range(FT)]
        ml_sb = [persist.tile([P, 512], BF16, tag=f"ml{c}", name=f"ml{c}") for c in range(NCH)]
        mr_sb = [persist.tile([P, 512], BF16, tag=f"mr{c}", name=f"mr{c}") for c in range(NCH)]
        for c in range(NCH):
            nc.sync.dma_start(ml_sb[c][:], t["ml"][c])
            nc.sync.dma_start(mr_sb[c][:], t["mr"][c])
        ones_f = persist.tile([P, 1], F32, tag="ones_f", name="ones_f")
        nc.vector.memset(ones_f[:], 1.0)
        ones_r = persist.tile([P, 1], F32R, tag="ones_r", name="ones_r")
        nc.scalar.activation(ones_r[:], ones_f[:], AF.Identity)
        from concourse.masks import make_identity
        ident = persist.tile([P, P], F32, tag="ident", name="ident")
        make_identity(nc, ident[:])
        hid_sb = persist.tile([P, 12], I32, tag="hid", name="hid")
        nc.sync.dma_start(hid_sb[:], t["halo_ids"][:])
        eps_sb = persist.tile([P, 1], F32, tag="eps", name="eps")
        nc.vector.memset(eps_sb[:], EPS)

        # ---- embedding + LN -> x^T ----
        with tc.tile_pool(name="emb_sb", bufs=1) as esb, \
             tc.tile_pool(name="emb_sb2", bufs=2) as esb2, \
             tc.tile_pool(name="emb_ps", bufs=2, space="PSUM") as eps:
            ids_sb = esb.tile([P, 12], I32, tag="ids", name="ids")
            nc.sync.dma_start(ids_sb[:], t["ids"][:])
            s_bc = esb.tile([P, HD], F32, tag="sbc", name="sbc")
            nc.sync.dma_start(s_bc[:], bass.AP(
                tensor=t["eln_s"].tensor, offset=0, ap=[[0, P], [1, HD]]))
            b_bc = esb.tile([P, HD], F32, tag="bbc", name="bbc")
            nc.sync.dma_start(b_bc[:], bass.AP(
                tensor=t["eln_b"].tensor, offset=0, ap=[[0, P], [1, HD]]))
            e = [esb.tile([P, HD], F32, tag=f"e{tt}", name=f"e{tt}") for tt in range(12)]
            for tt in range(12):
                nc.gpsimd.indirect_dma_start(
                    out=e[tt][:], out_offset=None, in_=t["emb"][:],
                    in_offset=bass.IndirectOffsetOnAxis(
                        ap=ids_sb[:, tt:tt + 1], axis=0))
                p_sb = esb2.tile([P, HD], F32, tag="pos", name="pos")
                nc.sync.dma_start(p_sb[:], t["pos"][tt * P:(tt + 1) * P, :])
                nc.vector.tensor_tensor(e[tt][:], e[tt][:], p_sb[:], op=ALU.add)
                stats = esb2.tile([P, 3, nc.vector.BN_STATS_DIM], F32,
                                  tag="bst", name="bst")
                er = e[tt][:].rearrange("p (g d) -> p g d", g=3)
                for g in range(3):
                    nc.vector.bn_stats(stats[:, g, :], er[:, g, :])
                mv = esb2.tile([P, nc.vector.BN_AGGR_DIM], F32, tag="bag", name="bag")
                nc.vector.bn_aggr(mv[:], stats[:])
                sd = esb2.tile([P, 1], F32, tag="bsd", name="bsd")
                nc.scalar.activation(sd[:], mv[:, 1:2], AF.Sqrt, bias=eps_sb[:])
                rstd = esb2.tile([P, 1], F32, tag="brstd", name="brstd")
                nc.vector.reciprocal(rstd[:], sd[:])
                nc.vector.tensor_scalar(out=e[tt][:], in0=e[tt][:],
                                        scalar1=mv[:, 0:1], scalar2=rstd[:],
                                        op0=ALU.subtract, op1=ALU.mult)
                nc.vector.tensor_tensor(e[tt][:], e[tt][:], s_bc[:], op=ALU.mult)
                nc.vector.tensor_tensor(e[tt][:], e[tt][:], b_bc[:], op=ALU.add)
            for ft in range(FT):
                tr = eps.tile([P, T_EXT], F32, tag="tr", name="tr")
                for tt in range(12):
                    nc.tensor.transpose(tr[:, tt * P:(tt + 1) * P],
                                        e[tt][:, ft * P:(ft + 1) * P], ident[:])
                nc.scalar.activation(x[ft][:], tr[:], AF.Identity)

        for l in range(n_layers):
            _layer(nc, tc, t, l, x, ml_sb, mr_sb, ones_r, eps_sb, hid_sb,
                   exchange=(l < n_layers - 1))

        for ft in range(FT):
            nc.gpsimd.dma_start(t["out"][ft], x[ft][:, W:W + T_OWN])


def _layer(nc, tc, t, l, x, ml_sb, mr_sb, ones_r, eps_sb, hid_sb, exchange):
    from contextlib import ExitStack
    with ExitStack() as ctx:
        lsb = ctx.enter_context(tc.tile_pool(name=f"lsb{l}", bufs=1))

        def bias_tile(name, n=FT):
            bt = lsb.tile([P, n], F32, tag=f"b_{name}", name=f"b_{name}")
            nc.sync.dma_start(bt[:], t[name][l])
            return bt
        bq_sb = bias_tile("bq"); bk_sb = bias_tile("bk"); bo_sb = bias_tile("bo")
        b1_sb = bias_tile("b1", FFT); b2_sb = bias_tile("b2")
        ls1_sb = bias_tile("ls1"); lb1_sb = bias_tile("lb1")
        ls2_sb = bias_tile("ls2"); lb2_sb = bias_tile("lb2")

        qT = [lsb.tile([P, T_OWN], F32R, tag=f"qT{i}", name=f"qT{i}") for i in range(FT)]
        kT = [lsb.tile([P, T_EXT], F32R, tag=f"kT{i}", name=f"kT{i}") for i in range(FT)]
        v = [lsb.tile([P, NH, 65], BF16, tag=f"v{i}", name=f"v{i}") for i in range(12)]
        o = [lsb.tile([P, T_OWN], BF16, tag=f"o{i}", name=f"o{i}") for i in range(FT)]
        r2 = [lsb.tile([P, 512], F32R, tag=f"r2_{i}", name=f"r2_{i}") for i in range(FT)]
        # aliases: qT tiles double as r1 (post-attention residual), kT's first
        # 1024 cols double as y (LN1 output) -- their producers are dead by then
        r1 = qT
        y = [kT[i][:, 0:T_OWN] for i in range(FT)]

        # ---- phase A: QKV projections ----
        with tc.tile_pool(name=f"pa_sb{l}", bufs=3) as asb, \
             tc.tile_pool(name=f"pa_wv{l}", bufs=1) as wvsb, \
             tc.tile_pool(name=f"pa_ps{l}", bufs=4, space="PSUM") as aps:
            for mt in range(FT):  # qT
                wq_sb = asb.tile([P, HD], F32R, tag="wqs", name="wqs")
                nc.sync.dma_start(wq_sb[:], t["wq"][l, mt])
                for h2 in range(2):
                    ps = aps.tile([P, 512], F32, tag="pp", name="pp")
                    for kt in range(FT):
                        nc.tensor.matmul(
                            ps[:], wq_sb[:, kt * P:(kt + 1) * P],
                            x[kt][:, W + h2 * 512:W + (h2 + 1) * 512],
                            start=(kt == 0), stop=(kt == FT - 1))
                    nc.scalar.activation(qT[mt][:, h2 * 512:(h2 + 1) * 512],
                                         ps[:], AF.Identity,
                                         bias=bq_sb[:, mt:mt + 1])
            for mt in range(FT):  # kT over ext tokens
                wk_sb = asb.tile([P, HD], F32R, tag="wks", name="wks")
                nc.sync.dma_start(wk_sb[:], t["wk"][l, mt])
                for h3 in range(3):
                    ps = aps.tile([P, 512], F32, tag="pp", name="pp")
                    for kt in range(FT):
                        nc.tensor.matmul(
                            ps[:], wk_sb[:, kt * P:(kt + 1) * P],
                            x[kt][:, h3 * 512:(h3 + 1) * 512],
                            start=(kt == 0), stop=(kt == FT - 1))
                    nc.scalar.activation(kT[mt][:, h3 * 512:(h3 + 1) * 512],
                                         ps[:], AF.Identity,
                                         bias=bk_sb[:, mt:mt + 1])
            # v natural [tok, d]: lhsT = x slice, rhs = wv strip
            wv_all = [wvsb.tile([P, HD], F32R, tag=f"wv{kt}", name=f"wv{kt}")
                      for kt in range(FT)]
            for kt in range(FT):
                nc.sync.dma_start(wv_all[kt][:],
                                  t["wv"][l, kt * P:(kt + 1) * P, :])
            for tt in range(12):
                for hf in range(2):
                    ps = aps.tile([P, 384], F32, tag="ppv", name="ppv", bufs=2)
                    for kt in range(FT):
                        nc.tensor.matmul(
                            ps[:], x[kt][:, tt * P:(tt + 1) * P],
                            wv_all[kt][:, hf * 384:(hf + 1) * 384],
                            start=(kt == 0), stop=(kt == FT - 1))
                    nc.scalar.activation(
                        v[tt][:, hf * 6:(hf + 1) * 6, 0:64],
                        ps[:].rearrange("p (h d) -> p h d", h=6), AF.Identity)
                nc.vector.memset(v[tt][:, :, 64:65], 1.0)

        # ---- phase B: banded attention ----
        with tc.tile_pool(name=f"pb_sb{l}", bufs=3) as bsb, \
             tc.tile_pool(name=f"pb_ps{l}", bufs=2, space="PSUM") as bps, \
             tc.tile_pool(name=f"pb_ps2{l}", bufs=2, space="PSUM") as bps2:
            for c in range(NCH):
                for h in range(NH):
                    ft, po = h // 2, (h % 2) * 64
                    sps = bps.tile([P, 6 * W], F32, tag="sps", name="sps")
                    for w in range(6):
                        nc.tensor.matmul(
                            sps[:, w * W:(w + 1) * W],
                            kT[ft][po:po + 64,
                                   c * W + w * P:c * W + (w + 1) * P],
                            qT[ft][po:po + 64, c * W:(c + 1) * W],
                            start=True, stop=True)
                    ex = bsb.tile([P, 6 * W], BF16, tag="ex", name="ex")
                    nc.scalar.activation(ex[:], sps[:], AF.Exp)
                    nc.vector.tensor_tensor(ex[:, 0:512], ex[:, 0:512],
                                            ml_sb[c][:], op=ALU.mult)
                    nc.vector.tensor_tensor(ex[:, 1024:1536], ex[:, 1024:1536],
                                            mr_sb[c][:], op=ALU.mult)
                    ops = bps2.tile([P, W], F32, tag="ops", name="ops")
                    for w in range(6):
                        nc.tensor.matmul(
                            ops[0:65, :],
                            v[c * 2 + w][:, h, :],
                            ex[:, w * W:(w + 1) * W],
                            start=(w == 0), stop=(w == 5))
                    rs = bsb.tile([1, W], F32, tag="rs", name="rs")
                    nc.vector.reciprocal(rs[:], ops[64:65, :])
                    rb = bsb.tile([64, W], F32, tag="rb", name="rb")
                    nc.gpsimd.partition_broadcast(rb[:], rs[:], channels=64)
                    nc.vector.tensor_tensor(
                        o[ft][po:po + 64, c * W:(c + 1) * W],
                        ops[0:64, :], rb[:], op=ALU.mult)

        # ---- phase C: O-proj + residual (r1 <- x + O@Wo + bo) ----
        with tc.tile_pool(name=f"pc_sb{l}", bufs=3) as csb, \
             tc.tile_pool(name=f"pc_ps{l}", bufs=4, space="PSUM") as cps:
            for mt in range(FT):
                wo_sb = csb.tile([P, HD], BF16, tag="wos", name="wos")
                nc.sync.dma_start(wo_sb[:], t["wo"][l, mt])
                for h2 in range(2):
                    ps = cps.tile([P, 512], F32, tag="ppo", name="ppo")
                    for kt in range(FT):
                        nc.tensor.matmul(
                            ps[:], wo_sb[:, kt * P:(kt + 1) * P],
                            o[kt][:, h2 * 512:(h2 + 1) * 512],
                            start=(kt == 0), stop=(kt == FT - 1))
                    nc.vector.scalar_tensor_tensor(
                        out=r1[mt][:, h2 * 512:(h2 + 1) * 512], in0=ps[:],
                        scalar=bo_sb[:, mt:mt + 1],
                        in1=x[mt][:, W + h2 * 512:W + (h2 + 1) * 512],
                        op0=ALU.add, op1=ALU.add)
        # ---- LN1 (per token-half): y <- LN(r1)*s+b ----
        for h2 in range(2):
            cs = slice(h2 * 512, (h2 + 1) * 512)
            with tc.tile_pool(name=f"ln1_sb{l}_{h2}", bufs=2) as l1sb, \
                 tc.tile_pool(name=f"ln1_ps{l}_{h2}", bufs=1,
                              space="PSUM") as l1ps:
                _ln_transposed(nc, l1sb, l1ps,
                               [r1[ft][:, cs] for ft in range(FT)],
                               [y[ft][:, cs] for ft in range(FT)],
                               ones_r, eps_sb, ls1_sb, lb1_sb, 512)

        # ---- phase D: FFN + residual + LN2 -> x middle ----
        for h2 in range(2):
            cs = slice(h2 * 512, (h2 + 1) * 512)
            csx = slice(W + h2 * 512, W + (h2 + 1) * 512)
            with ExitStack() as dctx:
                dsb = dctx.enter_context(
                    tc.tile_pool(name=f"pd_sb{l}_{h2}", bufs=3))
                zps = dctx.enter_context(
                    tc.tile_pool(name=f"pd_psz{l}_{h2}", bufs=1, space="PSUM"))
                fps = dctx.enter_context(
                    tc.tile_pool(name=f"pd_psf{l}_{h2}", bufs=2, space="PSUM"))
                zp = [zps.tile([P, 512], F32, tag=f"z{mt}", name=f"z{mt}") for mt in range(FT)]
                for ms in range(FFT):
                    w1_sb = dsb.tile([P, HD], F32R, tag="w1s", name="w1s")
                    nc.sync.dma_start(w1_sb[:], t["w1"][l, ms])
                    fp = fps.tile([P, 512], F32, tag="fp", name="fp")
                    for kt in range(FT):
                        nc.tensor.matmul(fp[:], w1_sb[:, kt * P:(kt + 1) * P],
                                         y[kt][:, cs], start=(kt == 0),
                                         stop=(kt == FT - 1))
                    f_sb = dsb.tile([P, 512], F32R, tag="fsb", name="fsb")
                    nc.scalar.activation(f_sb[:], fp[:], AF.Gelu,
                                         bias=b1_sb[:, ms:ms + 1])
                    w2_sb = dsb.tile([P, HD], F32R, tag="w2s", name="w2s")
                    nc.sync.dma_start(w2_sb[:],
                                      t["w2"][l, ms * P:(ms + 1) * P, :])
                    for mt in range(FT):
                        nc.tensor.matmul(zp[mt][:],
                                         w2_sb[:, mt * P:(mt + 1) * P],
                                         f_sb[:], start=(ms == 0),
                                         stop=(ms == FFT - 1))
                for mt in range(FT):
                    nc.vector.scalar_tensor_tensor(
                        out=r2[mt][:], in0=zp[mt][:],
                        scalar=b2_sb[:, mt:mt + 1], in1=y[mt][:, cs],
                        op0=ALU.add, op1=ALU.add)
            with tc.tile_pool(name=f"ln2_sb{l}_{h2}", bufs=2) as l2sb, \
                 tc.tile_pool(name=f"ln2_ps{l}_{h2}", bufs=1,
                              space="PSUM") as l2ps:
                _ln_transposed(nc, l2sb, l2ps,
                               [r2[ft][:] for ft in range(FT)],
                               [x[ft][:, csx] for ft in range(FT)],
                               ones_r, eps_sb, ls2_sb, lb2_sb, 512)

        # ---- phase E: halo exchange ----
        if exchange:
            with tc.tile_pool(name=f"pe_dram{l}", bufs=1, space="DRAM") as edram:
                b_in = edram.tile([2, FT, P, W], F32R, tag="bin", name="bin")
                b_out = edram.tile([4 * 2 * FT * P, W], F32R, tag="bout", name="bout")
                for ft in range(FT):
                    nc.sync.dma_start(b_in[0, ft], x[ft][:, W:2 * W])
                    nc.sync.dma_start(b_in[1, ft], x[ft][:, T_OWN:T_OWN + W])
                nc.gpsimd.collective_compute(
                    "AllGather", ALU.bypass,
                    replica_groups=[[0, 1, 2, 3], [4, 5, 6, 7]],
                    ins=[b_in[:].opt()], outs=[b_out[:].opt()])
                for side in range(2):
                    for ft in range(FT):
                        dst = (x[ft][:, 0:W] if side == 0
                               else x[ft][:, T_OWN + W:T_EXT])
                        nc.gpsimd.indirect_dma_start(
                            out=dst, out_offset=None, in_=b_out[:],
                            in_offset=bass.IndirectOffsetOnAxis(
                                ap=hid_sb[:, side * FT + ft:side * FT + ft + 1],
                                axis=0))


# ---------------- host side ----------------

def _blocked(w, n_k, n_m):
    """[n_k*128, n_m*128] -> [n_m, 128, n_k, 128] (lhsT strips by out-tile)."""
    return np.ascontiguousarray(
        w.reshape(n_k, P, n_m, P).transpose(2, 1, 0, 3))


def _bias_lay(b, n):
    return np.ascontiguousarray(b.reshape(n, P).T)


def prepare(inputs):
    """Build per-core in_maps from full inputs."""
    ids_full = np.asarray(inputs["input_ids"]).astype(np.int32)
    am = np.asarray(inputs["attention_mask"]).astype(np.int32)
    emb_word = np.asarray(inputs["emb_word"], dtype=np.float32)
    emb_pos = np.asarray(inputs["emb_pos"], dtype=np.float32)
    Wq = np.asarray(inputs["Wq"], np.float32) / np.sqrt(DH)
    bq = np.asarray(inputs["bq"], np.float32) / np.sqrt(DH)
    Wk = np.asarray(inputs["Wk"], np.float32)
    bk = np.asarray(inputs["bk"], np.float32)
    Wv = np.asarray(inputs["Wv"], np.float32)
    bv = np.asarray(inputs["bv"], np.float32)
    Wo = np.asarray(inputs["Wo"], np.float32)
    bo = np.asarray(inputs["bo"], np.float32)
    W1 = np.asarray(inputs["W1"], np.float32)
    b1 = np.asarray(inputs["b1"], np.float32)
    W2 = np.asarray(inputs["W2"], np.float32)
    b2 = np.asarray(inputs["b2"], np.float32)
    assert np.all(am == 1), "general attention_mask needs mid-tile masks too"

    shared = {
        "emb_word": emb_word,
        "eln_s": np.asarray(inputs["emb_ln_s"], np.float32),
        "eln_b": np.asarray(inputs["emb_ln_b"], np.float32),
        "wq": np.stack([_blocked(Wq[i], FT, FT) for i in range(L)]),
        "wk": np.stack([_blocked(Wk[i], FT, FT) for i in range(L)]),
        "wv": Wv,
        "wo": np.stack([_blocked(Wo[i], FT, FT) for i in range(L)]).astype(
            ml_dtypes.bfloat16),
        "w1": np.stack([_blocked(W1[i], FT, FFT) for i in range(L)]),
        "w2": W2,
        "bq": np.stack([_bias_lay(bq[i], FT) for i in range(L)]),
        "bk": np.stack([_bias_lay(bk[i], FT) for i in range(L)]),
        "bo": np.stack([_bias_lay(bv[i] @ Wo[i] + bo[i], FT)
                        for i in range(L)]),
        "b1": np.stack([_bias_lay(b1[i], FFT) for i in range(L)]),
        "b2": np.stack([_bias_lay(b2[i], FT) for i in range(L)]),
        "ls1": np.stack([_bias_lay(np.asarray(inputs["ln1_s"], np.float32)[i],
                                   FT) for i in range(L)]),
        "lb1": np.stack([_bias_lay(np.asarray(inputs["ln1_b"], np.float32)[i],
                                   FT) for i in range(L)]),
        "ls2": np.stack([_bias_lay(np.asarray(inputs["ln2_s"], np.float32)[i],
                                   FT) for i in range(L)]),
        "lb2": np.stack([_bias_lay(np.asarray(inputs["ln2_b"], np.float32)[i],
                                   FT) for i in range(L)]),
    }

    in_maps = []
    i_idx = np.arange(W)
    for core in range(N_CORES):
        b, sb = core // 4, core % 4
        s0 = sb * T_OWN
        ext_pos = np.clip(np.arange(s0 - W, s0 + T_OWN + W), 0, S - 1)
        m = dict(shared)
        m["ids"] = np.ascontiguousarray(
            ids_full[b, ext_pos].reshape(12, P).T)
        m["pos"] = np.ascontiguousarray(emb_pos[ext_pos])
        # masks: global chunk gc, window key j in [0,768), query i in [0,256):
        #   key_abs = gc*W - W + j ; allowed = |j - W - i| <= W
        #             & 0 <= key_abs < S & attention_mask[b, key_abs]
        mlm = np.zeros((NCH, P, 512), np.float32)
        mrm = np.zeros((NCH, P, 512), np.float32)
        for c in range(NCH):
            gc = sb * NCH + c
            for kt2 in range(2):
                for mm_, j0 in ((mlm, 0), (mrm, 512)):
                    j = j0 + kt2 * P + np.arange(P)[:, None]
                    key_abs = gc * W - W + j
                    ok = (np.abs(j - W - i_idx[None, :]) <= W)
                    ok &= (key_abs >= 0) & (key_abs < S)
                    ok &= am[b, np.clip(key_abs, 0, S - 1)] > 0
                    mm_[c, :, kt2 * W:(kt2 + 1) * W] = ok
        m["ml"] = mlm.astype(ml_dtypes.bfloat16)
        m["mr"] = mrm.astype(ml_dtypes.bfloat16)
        # halo row ids into the gathered [4, 2, FT, 128, W] row table
        hid = np.zeros((2, FT, P), np.int64)
        for side in range(2):
            nb = sb - 1 if side == 0 else sb + 1
            if 0 <= nb <= 3:
                osd = 1 - side  # left halo <- neighbor's right block
                for ft in range(FT):
                    hid[side, ft] = ((nb * 2 + osd) * FT + ft) * P \
                        + np.arange(P)
            else:
                for ft in range(FT):
                    hid[side, ft] = ((sb * 2 + side) * FT + ft) * P \
                        + np.arange(P)
        m["halo_ids"] = np.ascontiguousarray(
            hid.reshape(12, P).T.astype(np.int32))
        in_maps.append(m)
    return in_maps


_NC_CACHE = {}


def get_nc(n_layers=L):
    if n_layers not in _NC_CACHE:
        _NC_CACHE[n_layers] = build_nc(n_layers)
    return _NC_CACHE[n_layers]


def run(inputs, n_layers=L, trace=False):
    nc = get_nc(n_layers)
    in_maps = prepare(inputs)
    res = bass_utils.run_bass_kernel_spmd(
        nc, in_maps, core_ids=list(range(N_CORES)), trace=trace)
    outs = np.empty((B, S, HD), np.float32)
    for core in range(N_CORES):
        b, sb = core // 4, core % 4
        ot = res.results[core]["out"]  # [FT, 128, T_OWN]
        outs[b, sb * T_OWN:(sb + 1) * T_OWN] = ot.reshape(HD, T_OWN).T
    return outs, res


def kernel(**inputs) -> np.ndarray:
    out, _ = run(inputs)
    return out



# revision 15
# speedup vs baseline: 1.2515x; 1.2515x over previous
"""Longformer layer stack (4 layers, sliding-window attention) on 8 TRN2 cores.

Sharding: data-parallel over batch (2) x sequence-parallel (4 blocks of 1024
tokens). Each core computes its sequence block; the banded attention needs a
W=256 token halo, exchanged between neighboring blocks with an AllGather after
each layer (layers 0-2). Residual stream kept transposed ([dmodel, tokens]) in
bf16; all GEMMs bf16 (fp32 PSUM accumulate).

Scheduling notes:
- own-token work (Q, middle K/V chunks, middle attention chunks) is issued
  before halo-dependent work so the halo AllGather overlaps compute.
- LN uses rstd = exp(-0.5*ln(var+eps)) so the scalar engine stays on the
  natural_log_exp activation table set (avoids table thrash vs Sqrt).
- FFN gelu halves run back-to-back, LN2 halves after, to group table sets.
"""
import sys

sys.path.insert(0, '/opt/trn_rl_repo')

import numpy as np
import ml_dtypes

import concourse.bass as bass
import concourse.mybir as mybir
import concourse.tile as tile
from concourse import bacc
from concourse import bass_utils

F32 = mybir.dt.float32
BF16 = mybir.dt.bfloat16
I32 = mybir.dt.int32
AF = mybir.ActivationFunctionType
ALU = mybir.AluOpType

NH = 12          # heads
DH = 64          # head dim
HD = 768         # model dim
FF = 3072        # ffn dim
W = 256          # one-sided window
L = 4            # layers
B = 2
S = 4096
EPS = 1e-12
N_CORES = 8
T_OWN = 1024     # tokens per core
T_EXT = 1536     # with halos
FT = 6           # model-dim 128-tiles
FFT = 24         # ffn-dim 128-tiles
NCH = 4          # local chunks of 256 queries
P = 128


def _ln_transposed(nc, sb_stats, ps_stats, r_aps, out_aps, ones_b, eps_sb,
                   s_ap, b_ap, ncols):
    """LayerNorm over the partition (feature) axis of transposed bf16 tiles.

    r_aps: list of FT bf16 SBUF APs [128, ncols] (input; normalized IN PLACE
    up to the final scale/bias which lands in out_aps).
    out_aps[ft] <- LN(r)*s + b (bf16). s_ap/b_ap: [128, FT] f32 sbuf.
    rstd is computed as exp(-0.5*ln(var+eps)) to stay on the
    natural_log_exp scalar table set (same set as attention's Exp).
    """
    sumx = ps_stats.tile([1, ncols], F32, tag="lnsum", name="lnsum")
    sumsq = ps_stats.tile([1, ncols], F32, tag="lnsumsq", name="lnsumsq")
    nseg = (ncols + 511) // 512
    for ft in range(FT):
        sq = sb_stats.tile([P, ncols], BF16, tag="lnsq", name="lnsq")
        nc.vector.tensor_tensor(sq[:], r_aps[ft], r_aps[ft], op=ALU.mult)
        for j in range(nseg):
            cs = slice(j * 512, min((j + 1) * 512, ncols))
            nc.tensor.matmul(sumx[0:1, cs], ones_b[:, 0:1], r_aps[ft][:, cs],
                             start=(ft == 0), stop=(ft == FT - 1))
            nc.tensor.matmul(sumsq[0:1, cs], ones_b[:, 0:1], sq[:, cs],
                             start=(ft == 0), stop=(ft == FT - 1))
    mu_bf = sb_stats.tile([1, ncols], BF16, tag="lnmu", name="lnmu")
    nc.scalar.activation(mu_bf[:], sumx[:], AF.Identity, scale=1.0 / HD)
    musq = sb_stats.tile([1, ncols], F32, tag="lnmusq", name="lnmusq")
    nc.vector.tensor_tensor(musq[:], mu_bf[:], mu_bf[:], op=ALU.mult)
    var = sb_stats.tile([1, ncols], F32, tag="lnvar", name="lnvar")
    nc.vector.scalar_tensor_tensor(out=var[:], in0=sumsq[:], scalar=1.0 / HD,
                                   in1=musq[:], op0=ALU.mult, op1=ALU.subtract)
    lnv = sb_stats.tile([1, ncols], F32, tag="lnlnv", name="lnlnv")
    nc.scalar.activation(lnv[:], var[:], AF.Ln, bias=eps_sb[0:1, :])
    rstd_bf = sb_stats.tile([1, ncols], BF16, tag="lnrstd", name="lnrstd")
    nc.scalar.activation(rstd_bf[:], lnv[:], AF.Exp, scale=-0.5)
    mu_b = sb_stats.tile([P, ncols], BF16, tag="lnmub", name="lnmub")
    nc.gpsimd.partition_broadcast(mu_b[:], mu_bf[:], channels=P)
    rstd_b = sb_stats.tile([P, ncols], BF16, tag="lnrstdb", name="lnrstdb")
    nc.gpsimd.partition_broadcast(rstd_b[:], rstd_bf[:], channels=P)
    for ft in range(FT):
        nc.vector.tensor_tensor(r_aps[ft], r_aps[ft], mu_b[:], op=ALU.subtract)
        nc.vector.tensor_tensor(r_aps[ft], r_aps[ft], rstd_b[:], op=ALU.mult)
        nc.scalar.activation(out_aps[ft], r_aps[ft], AF.Identity,
                             scale=s_ap[:, ft:ft + 1], bias=b_ap[:, ft:ft + 1])


def build_nc(n_layers=L):
    nc = bacc.Bacc("TRN2", target_bir_lowering=False, debug=False,
                   num_devices=N_CORES)
    dt_ = nc.dram_tensor
    t = {}
    t["emb"] = dt_("emb_word", [32000, HD], F32, kind="ExternalInput").ap()
    t["ids"] = dt_("ids", [P, 12], I32, kind="ExternalInput").ap()
    t["pos"] = dt_("pos", [T_EXT, HD], F32, kind="ExternalInput").ap()
    t["wq"] = dt_("wq", [L, FT, P, FT, P], BF16, kind="ExternalInput").ap()
    t["wk"] = dt_("wk", [L, FT, P, FT, P], BF16, kind="ExternalInput").ap()
    t["wv"] = dt_("wv", [L, HD, HD], BF16, kind="ExternalInput").ap()
    t["wo"] = dt_("wo", [L, FT, P, FT, P], BF16, kind="ExternalInput").ap()
    t["w1"] = dt_("w1", [L, FFT, P, FT, P], BF16, kind="ExternalInput").ap()
    t["w2"] = dt_("w2", [L, FF, HD], BF16, kind="ExternalInput").ap()
    for nm in ["bq", "bk", "bo", "b2", "ls1", "lb1", "ls2", "lb2"]:
        t[nm] = dt_(nm, [L, P, FT], F32, kind="ExternalInput").ap()
    t["eln_s"] = dt_("eln_s", [P, FT], F32, kind="ExternalInput").ap()
    t["eln_b"] = dt_("eln_b", [P, FT], F32, kind="ExternalInput").ap()
    t["b1"] = dt_("b1", [L, P, FFT], F32, kind="ExternalInput").ap()
    t["ml"] = dt_("ml", [NCH, P, 512], BF16, kind="ExternalInput").ap()
    t["mr"] = dt_("mr", [NCH, P, 512], BF16, kind="ExternalInput").ap()
    t["halo_ids"] = dt_("halo_ids", [P, 12], I32, kind="ExternalInput").ap()
    t["out"] = dt_("out", [FT, P, T_OWN], F32, kind="ExternalOutput").ap()

    with tile.TileContext(nc) as tc:
        _build_body(nc, tc, n_layers, t)
    nc.compile()
    return nc


def _build_body(nc, tc, n_layers, t):
    from contextlib import ExitStack
    with ExitStack() as ctx:
        persist = ctx.enter_context(tc.tile_pool(name="persist", bufs=1))
        # residual stream, transposed, with halos: x[ft] = [128, T_EXT] bf16
        x = [persist.tile([P, T_EXT], BF16, tag=f"x{ft}", name=f"x{ft}") for ft in range(FT)]
        ml_sb = [persist.tile([P, 512], BF16, tag=f"ml{c}", name=f"ml{c}") for c in range(NCH)]
        mr_sb = [persist.tile([P, 512], BF16, tag=f"mr{c}", name=f"mr{c}") for c in range(NCH)]
        for c in range(NCH):
            nc.sync.dma_start(ml_sb[c][:], t["ml"][c])
            nc.sync.dma_start(mr_sb[c][:], t["mr"][c])
        ones_f = persist.tile([P, 1], F32, tag="ones_f", name="ones_f")
        nc.vector.memset(ones_f[:], 1.0)
        ones_b = persist.tile([P, 1], BF16, tag="ones_b", name="ones_b")
        nc.scalar.activation(ones_b[:], ones_f[:], AF.Identity)
        from concourse.masks import make_identity
        ident_b = persist.tile([P, P], BF16, tag="identb", name="identb")
        make_identity(nc, ident_b[:])
        hid_sb = persist.tile([P, 12], I32, tag="hid", name="hid")
        nc.sync.dma_start(hid_sb[:], t["halo_ids"][:])
        eps_sb = persist.tile([P, 1], F32, tag="eps", name="eps")
        nc.vector.memset(eps_sb[:], EPS)

        # ---- embedding (+pos) -> transpose -> LN -> x^T ----
        with ExitStack() as ectx:
            esb = ectx.enter_context(tc.tile_pool(name="emb_sb", bufs=1))
            esb2 = ectx.enter_context(tc.tile_pool(name="emb_sb2", bufs=2))
            ids_sb = esb.tile([P, 12], I32, tag="ids", name="ids")
            nc.sync.dma_start(ids_sb[:], t["ids"][:])
            e = [esb.tile([P, HD], F32, tag=f"e{tt}", name=f"e{tt}") for tt in range(12)]
            p_sb = [esb.tile([P, HD], F32, tag=f"p{tt}", name=f"p{tt}") for tt in range(12)]
            ebf = [esb.tile([P, HD], BF16, tag=f"eb{tt}", name=f"eb{tt}") for tt in range(12)]
            for tt in range(12):
                nc.gpsimd.indirect_dma_start(
                    out=e[tt][:], out_offset=None, in_=t["emb"][:],
                    in_offset=bass.IndirectOffsetOnAxis(
                        ap=ids_sb[:, tt:tt + 1], axis=0))
                nc.sync.dma_start(p_sb[tt][:], t["pos"][tt * P:(tt + 1) * P, :])
            for tt in range(12):
                nc.vector.tensor_tensor(ebf[tt][:], e[tt][:], p_sb[tt][:],
                                        op=ALU.add)
            with tc.tile_pool(name="emb_ps", bufs=2, space="PSUM") as eps_ps:
                for ft in range(FT):
                    tr = eps_ps.tile([P, T_EXT], BF16, tag="tr", name="tr")
                    for tt in range(12):
                        nc.tensor.transpose(tr[:, tt * P:(tt + 1) * P],
                                            ebf[tt][:, ft * P:(ft + 1) * P],
                                            ident_b[:])
                    nc.vector.tensor_copy(x[ft][:], tr[:])
            eln_s = esb.tile([P, FT], F32, tag="elns", name="elns")
            nc.sync.dma_start(eln_s[:], t["eln_s"][:])
            eln_b = esb.tile([P, FT], F32, tag="elnb", name="elnb")
            nc.sync.dma_start(eln_b[:], t["eln_b"][:])
            with tc.tile_pool(name="emb_lnsb", bufs=2) as lsb_e, \
                 tc.tile_pool(name="emb_lnps", bufs=1, space="PSUM") as lps_e:
                _ln_transposed(nc, lsb_e, lps_e,
                               [x[ft][:] for ft in range(FT)],
                               [x[ft][:] for ft in range(FT)],
                               ones_b, eps_sb, eln_s, eln_b, T_EXT)

        # final-layer LN2 writes fp32 straight into xout (skips bf16 rounding
        # of the kernel output)
        xout = [persist.tile([P, T_OWN], F32, tag=f"xo{ft}", name=f"xo{ft}")
                for ft in range(FT)]
        for l in range(n_layers):
            _layer(nc, tc, t, l, x, ml_sb, mr_sb, ones_b, eps_sb, hid_sb,
                   exchange=(l < n_layers - 1), xout=xout,
                   final=(l == n_layers - 1))
        if n_layers == 0:
            for ft in range(FT):
                nc.vector.tensor_copy(xout[ft][:], x[ft][:, W:W + T_OWN])

        for ft in range(FT):
            nc.gpsimd.dma_start(t["out"][ft], xout[ft][:])


def _layer(nc, tc, t, l, x, ml_sb, mr_sb, ones_b, eps_sb, hid_sb, exchange,
           xout=None, final=False):
    from contextlib import ExitStack
    with ExitStack() as ctx:
        lsb = ctx.enter_context(tc.tile_pool(name=f"lsb{l}", bufs=1))

        def bias_tile(name, n=FT):
            bt = lsb.tile([P, n], F32, tag=f"b_{name}", name=f"b_{name}")
            nc.sync.dma_start(bt[:], t[name][l])
            return bt
        bq_sb = bias_tile("bq"); bk_sb = bias_tile("bk"); bo_sb = bias_tile("bo")
        b1_sb = bias_tile("b1", FFT); b2_sb = bias_tile("b2")
        ls1_sb = bias_tile("ls1"); lb1_sb = bias_tile("lb1")
        ls2_sb = bias_tile("ls2"); lb2_sb = bias_tile("lb2")

        qT = [lsb.tile([P, T_OWN], BF16, tag=f"qT{i}", name=f"qT{i}") for i in range(FT)]
        kT = [lsb.tile([P, T_EXT], BF16, tag=f"kT{i}", name=f"kT{i}") for i in range(FT)]
        v = [lsb.tile([P, NH, 65], BF16, tag=f"v{i}", name=f"v{i}") for i in range(12)]
        o = [lsb.tile([P, T_OWN], BF16, tag=f"o{i}", name=f"o{i}") for i in range(FT)]
        r2 = [lsb.tile([P, T_OWN], BF16, tag=f"r2_{i}", name=f"r2_{i}") for i in range(FT)]
        # aliases: qT tiles double as r1 (post-attention residual), kT's first
        # 1024 cols double as y (LN1 output) -- their producers are dead by then
        r1 = qT
        y = [kT[i][:, 0:T_OWN] for i in range(FT)]

        # ---- phase A: QKV projections (own-token work first; halo last) ----
        with tc.tile_pool(name=f"pa_sb{l}", bufs=3) as asb, \
             tc.tile_pool(name=f"pa_w{l}", bufs=1) as wsb, \
             tc.tile_pool(name=f"pa_ps{l}", bufs=4, space="PSUM") as aps:
            wk_all = [wsb.tile([P, HD], BF16, tag=f"wk{kt}", name=f"wk{kt}")
                      for kt in range(FT)]
            wv_all = [wsb.tile([P, HD], BF16, tag=f"wv{kt}", name=f"wv{kt}")
                      for kt in range(FT)]
            for kt in range(FT):
                nc.sync.dma_start(wk_all[kt][:], t["wk"][l, kt])
                nc.sync.dma_start(wv_all[kt][:],
                                  t["wv"][l, kt * P:(kt + 1) * P, :])
            for mt in range(FT):  # qT (own tokens only)
                wq_sb = asb.tile([P, HD], BF16, tag="wqs", name="wqs")
                nc.sync.dma_start(wq_sb[:], t["wq"][l, mt])
                for h2 in range(2):
                    ps = aps.tile([P, 512], F32, tag="pp", name="pp")
                    for kt in range(FT):
                        nc.tensor.matmul(
                            ps[:], wq_sb[:, kt * P:(kt + 1) * P],
                            x[kt][:, W + h2 * 512:W + (h2 + 1) * 512],
                            start=(kt == 0), stop=(kt == FT - 1))
                    nc.scalar.activation(qT[mt][:, h2 * 512:(h2 + 1) * 512],
                                         ps[:], AF.Identity,
                                         bias=bq_sb[:, mt:mt + 1])

            def k_chunk(mt, c0, c1):
                ps = aps.tile([P, 512], F32, tag="pp", name="pp")
                for kt in range(FT):
                    nc.tensor.matmul(
                        ps[:, 0:c1 - c0], wk_all[mt][:, kt * P:(kt + 1) * P],
                        x[kt][:, c0:c1], start=(kt == 0), stop=(kt == FT - 1))
                nc.scalar.activation(kT[mt][:, c0:c1], ps[:, 0:c1 - c0],
                                     AF.Identity, bias=bk_sb[:, mt:mt + 1])

            def v_tt(tt):
                for hf in range(2):
                    ps = aps.tile([P, 384], F32, tag="ppv", name="ppv", bufs=2)
                    for kt in range(FT):
                        nc.tensor.matmul(
                            ps[:], x[kt][:, tt * P:(tt + 1) * P],
                            wv_all[kt][:, hf * 384:(hf + 1) * 384],
                            start=(kt == 0), stop=(kt == FT - 1))
                    nc.scalar.activation(
                        v[tt][:, hf * 6:(hf + 1) * 6, 0:64],
                        ps[:].rearrange("p (h d) -> p h d", h=6), AF.Identity)
                nc.vector.memset(v[tt][:, :, 64:65], 1.0)

            for mt in range(FT):   # K own chunks
                k_chunk(mt, W, W + 512)
                k_chunk(mt, W + 512, W + 1024)
            for tt in range(2, 10):  # V own tokens
                v_tt(tt)
            for mt in range(FT):   # K halo chunks (wait on AllGather)
                k_chunk(mt, 0, W)
                k_chunk(mt, W + 1024, T_EXT)
            for tt in (0, 1, 10, 11):  # V halo tokens
                v_tt(tt)

        # ---- phase B: banded attention (own chunks first) ----
        with tc.tile_pool(name=f"pb_sb{l}", bufs=3) as bsb, \
             tc.tile_pool(name=f"pb_ps{l}", bufs=2, space="PSUM") as bps, \
             tc.tile_pool(name=f"pb_ps2{l}", bufs=2, space="PSUM") as bps2:
            for c in (1, 2, 0, 3):
                for h in range(NH):
                    ft, po = h // 2, (h % 2) * 64
                    sps = bps.tile([P, 6 * W], F32, tag="sps", name="sps")
                    for w in range(6):
                        nc.tensor.matmul(
                            sps[:, w * W:(w + 1) * W],
                            kT[ft][po:po + 64,
                                   c * W + w * P:c * W + (w + 1) * P],
                            qT[ft][po:po + 64, c * W:(c + 1) * W],
                            start=True, stop=True)
                    ex = bsb.tile([P, 6 * W], BF16, tag="ex", name="ex")
                    nc.scalar.activation(ex[:], sps[:], AF.Exp)
                    nc.vector.tensor_tensor(ex[:, 0:512], ex[:, 0:512],
                                            ml_sb[c][:], op=ALU.mult)
                    nc.vector.tensor_tensor(ex[:, 1024:1536], ex[:, 1024:1536],
                                            mr_sb[c][:], op=ALU.mult)
                    ops = bps2.tile([P, W], F32, tag="ops", name="ops")
                    for w in range(6):
                        nc.tensor.matmul(
                            ops[0:65, :],
                            v[c * 2 + w][:, h, :],
                            ex[:, w * W:(w + 1) * W],
                            start=(w == 0), stop=(w == 5))
                    rs0 = bsb.tile([1, W], F32, tag="rs0", name="rs0")
                    nc.vector.tensor_copy(rs0[:], ops[64:65, :])
                    rs = bsb.tile([1, W], F32, tag="rs", name="rs")
                    nc.vector.reciprocal_approx_fast(rs[:], rs0[:])
                    rb = bsb.tile([64, W], F32, tag="rb", name="rb")
                    nc.gpsimd.partition_broadcast(rb[:], rs[:], channels=64)
                    nc.vector.tensor_tensor(
                        o[ft][po:po + 64, c * W:(c + 1) * W],
                        ops[0:64, :], rb[:], op=ALU.mult)

        # ---- phase C: O-proj + residual (r1 <- x + O@Wo + bo), h2-major ----
        with tc.tile_pool(name=f"pc_sb{l}", bufs=1) as csb, \
             tc.tile_pool(name=f"pc_ps{l}", bufs=4, space="PSUM") as cps:
            wo_all = [csb.tile([P, HD], BF16, tag=f"wo{mt}", name=f"wo{mt}")
                      for mt in range(FT)]
            for mt in range(FT):
                nc.sync.dma_start(wo_all[mt][:], t["wo"][l, mt])
            for h2 in range(2):
                for mt in range(FT):
                    ps = cps.tile([P, 512], F32, tag="ppo", name="ppo")
                    for kt in range(FT):
                        nc.tensor.matmul(
                            ps[:], wo_all[mt][:, kt * P:(kt + 1) * P],
                            o[kt][:, h2 * 512:(h2 + 1) * 512],
                            start=(kt == 0), stop=(kt == FT - 1))
                    nc.vector.scalar_tensor_tensor(
                        out=r1[mt][:, h2 * 512:(h2 + 1) * 512], in0=ps[:],
                        scalar=bo_sb[:, mt:mt + 1],
                        in1=x[mt][:, W + h2 * 512:W + (h2 + 1) * 512],
                        op0=ALU.add, op1=ALU.add)
                # LN1 for this half: y <- LN(r1)*s+b
                cs = slice(h2 * 512, (h2 + 1) * 512)
                with tc.tile_pool(name=f"ln1_sb{l}_{h2}", bufs=2) as l1sb, \
                     tc.tile_pool(name=f"ln1_ps{l}_{h2}", bufs=1,
                                  space="PSUM") as l1ps:
                    _ln_transposed(nc, l1sb, l1ps,
                                   [r1[ft][:, cs] for ft in range(FT)],
                                   [y[ft][:, cs] for ft in range(FT)],
                                   ones_b, eps_sb, ls1_sb, lb1_sb, 512)

        # ---- phase D: FFN + residual + LN2 -> x middle (per token-half);
        # halo edge blocks are DMA'd to DRAM as soon as their LN2 half lands
        edram = ctx.enter_context(
            tc.tile_pool(name=f"pe_dram{l}", bufs=1, space="DRAM")) \
            if exchange else None
        if exchange:
            b_in = edram.tile([2, FT, P, W], BF16, tag="bin", name="bin")
            b_out = edram.tile([4 * 2 * FT * P, W], BF16, tag="bout",
                               name="bout")
        for h2 in range(2):
            cs = slice(h2 * 512, (h2 + 1) * 512)
            csx = slice(W + h2 * 512, W + (h2 + 1) * 512)
            with ExitStack() as dctx:
                dsb = dctx.enter_context(
                    tc.tile_pool(name=f"pd_sb{l}_{h2}", bufs=3))
                zps = dctx.enter_context(
                    tc.tile_pool(name=f"pd_psz{l}_{h2}", bufs=1, space="PSUM"))
                fps = dctx.enter_context(
                    tc.tile_pool(name=f"pd_psf{l}_{h2}", bufs=2, space="PSUM"))
                zp = [zps.tile([P, 512], F32, tag=f"z{mt}", name=f"z{mt}") for mt in range(FT)]
                for ms in range(FFT):
                    w1_sb = dsb.tile([P, HD], BF16, tag="w1s", name="w1s")
                    nc.sync.dma_start(w1_sb[:], t["w1"][l, ms])
                    fp = fps.tile([P, 512], F32, tag="fp", name="fp")
                    for kt in range(FT):
                        nc.tensor.matmul(fp[:], w1_sb[:, kt * P:(kt + 1) * P],
                                         y[kt][:, cs], start=(kt == 0),
                                         stop=(kt == FT - 1))
                    f_sb = dsb.tile([P, 512], BF16, tag="fsb", name="fsb")
                    nc.scalar.activation(f_sb[:], fp[:], AF.Gelu,
                                         bias=b1_sb[:, ms:ms + 1])
                    w2_sb = dsb.tile([P, HD], BF16, tag="w2s", name="w2s")
                    nc.sync.dma_start(w2_sb[:],
                                      t["w2"][l, ms * P:(ms + 1) * P, :])
                    for mt in range(FT):
                        nc.tensor.matmul(zp[mt][:],
                                         w2_sb[:, mt * P:(mt + 1) * P],
                                         f_sb[:], start=(ms == 0),
                                         stop=(ms == FFT - 1))
                for mt in range(FT):
                    nc.vector.scalar_tensor_tensor(
                        out=r2[mt][:, cs], in0=zp[mt][:],
                        scalar=b2_sb[:, mt:mt + 1], in1=y[mt][:, cs],
                        op0=ALU.add, op1=ALU.add)
            ln2_out = ([xout[ft][:, cs] for ft in range(FT)] if final
                       else [x[ft][:, csx] for ft in range(FT)])
            with tc.tile_pool(name=f"ln2_sb{l}_{h2}", bufs=2) as l2sb, \
                 tc.tile_pool(name=f"ln2_ps{l}_{h2}", bufs=1,
                              space="PSUM") as l2ps:
                _ln_transposed(nc, l2sb, l2ps,
                               [r2[ft][:, cs] for ft in range(FT)],
                               ln2_out,
                               ones_b, eps_sb, ls2_sb, lb2_sb, 512)
            if exchange:
                for ft in range(FT):
                    if h2 == 0:
                        nc.sync.dma_start(b_in[0, ft], x[ft][:, W:2 * W])
                    else:
                        nc.sync.dma_start(b_in[1, ft],
                                          x[ft][:, T_OWN:T_OWN + W])

        # ---- phase E: halo exchange ----
        if exchange:
            nc.gpsimd.collective_compute(
                "AllGather", ALU.bypass,
                replica_groups=[[0, 1, 2, 3], [4, 5, 6, 7]],
                ins=[b_in[:].opt()], outs=[b_out[:].opt()])
            for side in range(2):
                for ft in range(FT):
                    dst = (x[ft][:, 0:W] if side == 0
                           else x[ft][:, T_OWN + W:T_EXT])
                    nc.gpsimd.indirect_dma_start(
                        out=dst, out_offset=None, in_=b_out[:],
                        in_offset=bass.IndirectOffsetOnAxis(
                            ap=hid_sb[:, side * FT + ft:side * FT + ft + 1],
                            axis=0))


# ---------------- host side ----------------

def _blocked(w, n_k, n_m):
    """[n_k*128, n_m*128] -> [n_m, 128, n_k, 128] (lhsT strips by out-tile)."""
    return np.ascontiguousarray(
        w.reshape(n_k, P, n_m, P).transpose(2, 1, 0, 3))


def _bias_lay(b, n):
    return np.ascontiguousarray(b.reshape(n, P).T)


def prepare(inputs):
    """Build per-core in_maps from full inputs."""
    bf = ml_dtypes.bfloat16
    ids_full = np.asarray(inputs["input_ids"]).astype(np.int32)
    am = np.asarray(inputs["attention_mask"]).astype(np.int32)
    emb_word = np.asarray(inputs["emb_word"], dtype=np.float32)
    emb_pos = np.asarray(inputs["emb_pos"], dtype=np.float32)
    Wq = np.asarray(inputs["Wq"], np.float32) / np.sqrt(DH)
    bq = np.asarray(inputs["bq"], np.float32) / np.sqrt(DH)
    Wk = np.asarray(inputs["Wk"], np.float32)
    bk = np.asarray(inputs["bk"], np.float32)
    Wv = np.asarray(inputs["Wv"], np.float32)
    bv = np.asarray(inputs["bv"], np.float32)
    Wo = np.asarray(inputs["Wo"], np.float32)
    bo = np.asarray(inputs["bo"], np.float32)
    W1 = np.asarray(inputs["W1"], np.float32)
    b1 = np.asarray(inputs["b1"], np.float32)
    W2 = np.asarray(inputs["W2"], np.float32)
    b2 = np.asarray(inputs["b2"], np.float32)
    assert np.all(am == 1), "general attention_mask needs mid-tile masks too"

    shared = {
        "emb_word": emb_word,
        "eln_s": _bias_lay(np.asarray(inputs["emb_ln_s"], np.float32), FT),
        "eln_b": _bias_lay(np.asarray(inputs["emb_ln_b"], np.float32), FT),
        "wq": np.stack([_blocked(Wq[i], FT, FT) for i in range(L)]).astype(bf),
        "wk": np.stack([_blocked(Wk[i], FT, FT) for i in range(L)]).astype(bf),
        "wv": Wv.astype(bf),
        "wo": np.stack([_blocked(Wo[i], FT, FT) for i in range(L)]).astype(bf),
        "w1": np.stack([_blocked(W1[i], FT, FFT) for i in range(L)]).astype(bf),
        "w2": W2.astype(bf),
        "bq": np.stack([_bias_lay(bq[i], FT) for i in range(L)]),
        "bk": np.stack([_bias_lay(bk[i], FT) for i in range(L)]),
        "bo": np.stack([_bias_lay(bv[i] @ Wo[i] + bo[i], FT)
                        for i in range(L)]),
        "b1": np.stack([_bias_lay(b1[i], FFT) for i in range(L)]),
        "b2": np.stack([_bias_lay(b2[i], FT) for i in range(L)]),
        "ls1": np.stack([_bias_lay(np.asarray(inputs["ln1_s"], np.float32)[i],
                                   FT) for i in range(L)]),
        "lb1": np.stack([_bias_lay(np.asarray(inputs["ln1_b"], np.float32)[i],
                                   FT) for i in range(L)]),
        "ls2": np.stack([_bias_lay(np.asarray(inputs["ln2_s"], np.float32)[i],
                                   FT) for i in range(L)]),
        "lb2": np.stack([_bias_lay(np.asarray(inputs["ln2_b"], np.float32)[i],
                                   FT) for i in range(L)]),
    }

    in_maps = []
    i_idx = np.arange(W)
    for core in range(N_CORES):
        b, sb = core // 4, core % 4
        s0 = sb * T_OWN
        ext_pos = np.clip(np.arange(s0 - W, s0 + T_OWN + W), 0, S - 1)
        m = dict(shared)
        m["ids"] = np.ascontiguousarray(
            ids_full[b, ext_pos].reshape(12, P).T)
        m["pos"] = np.ascontiguousarray(emb_pos[ext_pos])
        # masks: global chunk gc, window key j in [0,768), query i in [0,256):
        #   key_abs = gc*W - W + j ; allowed = |j - W - i| <= W
        #             & 0 <= key_abs < S & attention_mask[b, key_abs]
        mlm = np.zeros((NCH, P, 512), np.float32)
        mrm = np.zeros((NCH, P, 512), np.float32)
        for c in range(NCH):
            gc = sb * NCH + c
            for kt2 in range(2):
                for mm_, j0 in ((mlm, 0), (mrm, 512)):
                    j = j0 + kt2 * P + np.arange(P)[:, None]
                    key_abs = gc * W - W + j
                    ok = (np.abs(j - W - i_idx[None, :]) <= W)
                    ok &= (key_abs >= 0) & (key_abs < S)
                    ok &= am[b, np.clip(key_abs, 0, S - 1)] > 0
                    mm_[c, :, kt2 * W:(kt2 + 1) * W] = ok
        m["ml"] = mlm.astype(bf)
        m["mr"] = mrm.astype(bf)
        # halo row ids into the gathered [4, 2, FT, 128, W] row table
        hid = np.zeros((2, FT, P), np.int64)
        for side in range(2):
            nb = sb - 1 if side == 0 else sb + 1
            if 0 <= nb <= 3:
                osd = 1 - side  # left halo <- neighbor's right block
                for ft in range(FT):
                    hid[side, ft] = ((nb * 2 + osd) * FT + ft) * P \
                        + np.arange(P)
            else:
                for ft in range(FT):
                    hid[side, ft] = ((sb * 2 + side) * FT + ft) * P \
                        + np.arange(P)
        m["halo_ids"] = np.ascontiguousarray(
            hid.reshape(12, P).T.astype(np.int32))
        in_maps.append(m)
    return in_maps


_NC_CACHE = {}


def get_nc(n_layers=L):
    if n_layers not in _NC_CACHE:
        _NC_CACHE[n_layers] = build_nc(n_layers)
    return _NC_CACHE[n_layers]


def run(inputs, n_layers=L, trace=False):
    nc = get_nc(n_layers)
    in_maps = prepare(inputs)
    res = bass_utils.run_bass_kernel_spmd(
        nc, in_maps, core_ids=list(range(N_CORES)), trace=trace)
    outs = np.empty((B, S, HD), np.float32)
    for core in range(N_CORES):
        b, sb = core // 4, core % 4
        ot = res.results[core]["out"]  # [FT, 128, T_OWN] f32
        outs[b, sb * T_OWN:(sb + 1) * T_OWN] = \
            np.asarray(ot, dtype=np.float32).reshape(HD, T_OWN).T
    return outs, res


def kernel(**inputs) -> np.ndarray:
    out, _ = run(inputs)
    return out


# revision 21
# speedup vs baseline: 1.2810x; 1.0236x over previous
"""Longformer layer stack (4 layers, sliding-window attention) on 8 TRN2 cores.

Sharding: data-parallel over batch (2) x sequence-parallel (4 blocks of 1024
tokens). Each core computes its sequence block; the banded attention needs a
W=256 token halo, exchanged between neighboring blocks with an AllGather after
each layer (layers 0-2). Residual stream kept transposed ([dmodel, tokens]) in
bf16; all GEMMs bf16 (fp32 PSUM accumulate).

Scheduling notes:
- own-token work (Q, middle K/V chunks, middle attention chunks) is issued
  before halo-dependent work so the halo AllGather overlaps compute.
- LN uses rstd = exp(-0.5*ln(var+eps)) so the scalar engine stays on the
  natural_log_exp activation table set (avoids table thrash vs Sqrt).
- FFN gelu halves run back-to-back, LN2 halves after, to group table sets.
"""
import sys

sys.path.insert(0, '/opt/trn_rl_repo')

import numpy as np
import ml_dtypes

import concourse.bass as bass
import concourse.mybir as mybir
import concourse.tile as tile
from concourse import bacc
from concourse import bass_utils

F32 = mybir.dt.float32
BF16 = mybir.dt.bfloat16
I32 = mybir.dt.int32
AF = mybir.ActivationFunctionType
ALU = mybir.AluOpType

NH = 12          # heads
DH = 64          # head dim
HD = 768         # model dim
FF = 3072        # ffn dim
W = 256          # one-sided window
L = 4            # layers
B = 2
S = 4096
EPS = 1e-12
N_CORES = 8
T_OWN = 1024     # tokens per core
T_EXT = 1536     # with halos
FT = 6           # model-dim 128-tiles
FFT = 24         # ffn-dim 128-tiles
NCH = 4          # local chunks of 256 queries
P = 128


def _ln_transposed(nc, sb_stats, ps_stats, r_aps, out_aps, ones_b, eps_sb,
                   s_ap, b_ap, ncols):
    """LayerNorm over the partition (feature) axis of transposed bf16 tiles.

    r_aps: list of FT bf16 SBUF APs [128, ncols] (input; normalized IN PLACE
    up to the final scale/bias which lands in out_aps).
    out_aps[ft] <- LN(r)*s + b (bf16). s_ap/b_ap: [128, FT] f32 sbuf.
    rstd is computed as exp(-0.5*ln(var+eps)) to stay on the
    natural_log_exp scalar table set (same set as attention's Exp).
    """
    sumx = ps_stats.tile([1, ncols], F32, tag="lnsum", name="lnsum")
    sumsq = ps_stats.tile([1, ncols], F32, tag="lnsumsq", name="lnsumsq")
    nseg = (ncols + 511) // 512
    for ft in range(FT):
        sq = sb_stats.tile([P, ncols], BF16, tag="lnsq", name="lnsq")
        nc.vector.tensor_tensor(sq[:], r_aps[ft], r_aps[ft], op=ALU.mult)
        for j in range(nseg):
            cs = slice(j * 512, min((j + 1) * 512, ncols))
            nc.tensor.matmul(sumx[0:1, cs], ones_b[:, 0:1], r_aps[ft][:, cs],
                             start=(ft == 0), stop=(ft == FT - 1))
            nc.tensor.matmul(sumsq[0:1, cs], ones_b[:, 0:1], sq[:, cs],
                             start=(ft == 0), stop=(ft == FT - 1))
    mu_bf = sb_stats.tile([1, ncols], BF16, tag="lnmu", name="lnmu")
    nc.scalar.activation(mu_bf[:], sumx[:], AF.Identity, scale=1.0 / HD)
    musq = sb_stats.tile([1, ncols], F32, tag="lnmusq", name="lnmusq")
    nc.vector.tensor_tensor(musq[:], mu_bf[:], mu_bf[:], op=ALU.mult)
    var = sb_stats.tile([1, ncols], F32, tag="lnvar", name="lnvar")
    nc.vector.scalar_tensor_tensor(out=var[:], in0=sumsq[:], scalar=1.0 / HD,
                                   in1=musq[:], op0=ALU.mult, op1=ALU.subtract)
    sd = sb_stats.tile([1, ncols], F32, tag="lnsd", name="lnsd")
    nc.scalar.activation(sd[:], var[:], AF.Sqrt, bias=eps_sb[0:1, :])
    rstd = sb_stats.tile([1, ncols], F32, tag="lnrstdf", name="lnrstdf")
    nc.vector.reciprocal_approx_fast(rstd[:], sd[:])
    rstd_bf = sb_stats.tile([1, ncols], BF16, tag="lnrstd", name="lnrstd")
    nc.vector.tensor_copy(rstd_bf[:], rstd[:])
    mu_b = sb_stats.tile([P, ncols], BF16, tag="lnmub", name="lnmub")
    nc.gpsimd.partition_broadcast(mu_b[:], mu_bf[:], channels=P)
    rstd_b = sb_stats.tile([P, ncols], BF16, tag="lnrstdb", name="lnrstdb")
    nc.gpsimd.partition_broadcast(rstd_b[:], rstd_bf[:], channels=P)
    for ft in range(FT):
        nc.vector.tensor_tensor(r_aps[ft], r_aps[ft], mu_b[:], op=ALU.subtract)
        nc.vector.tensor_tensor(r_aps[ft], r_aps[ft], rstd_b[:], op=ALU.mult)
        nc.scalar.activation(out_aps[ft], r_aps[ft], AF.Identity,
                             scale=s_ap[:, ft:ft + 1], bias=b_ap[:, ft:ft + 1])


def build_nc(n_layers=L):
    nc = bacc.Bacc("TRN2", target_bir_lowering=False, debug=False,
                   num_devices=N_CORES)
    dt_ = nc.dram_tensor
    t = {}
    t["emb"] = dt_("emb_word", [32000, HD], F32, kind="ExternalInput").ap()
    t["ids"] = dt_("ids", [P, 12], I32, kind="ExternalInput").ap()
    t["pos"] = dt_("pos", [T_EXT, HD], F32, kind="ExternalInput").ap()
    t["wq"] = dt_("wq", [L, FT, P, FT, P], BF16, kind="ExternalInput").ap()
    t["wk"] = dt_("wk", [L, FT, P, FT, P], BF16, kind="ExternalInput").ap()
    t["wv"] = dt_("wv", [L, HD, HD], BF16, kind="ExternalInput").ap()
    t["wo"] = dt_("wo", [L, FT, P, FT, P], BF16, kind="ExternalInput").ap()
    t["w1"] = dt_("w1", [L, FFT, P, FT, P], BF16, kind="ExternalInput").ap()
    t["w2"] = dt_("w2", [L, FF, HD], BF16, kind="ExternalInput").ap()
    for nm in ["bq", "bk", "bo", "b2", "ls1", "lb1", "ls2", "lb2"]:
        t[nm] = dt_(nm, [L, P, FT], F32, kind="ExternalInput").ap()
    t["eln_s"] = dt_("eln_s", [P, FT], F32, kind="ExternalInput").ap()
    t["eln_b"] = dt_("eln_b", [P, FT], F32, kind="ExternalInput").ap()
    t["b1"] = dt_("b1", [L, P, FFT], F32, kind="ExternalInput").ap()
    t["ml"] = dt_("ml", [NCH, P, 512], BF16, kind="ExternalInput").ap()
    t["mr"] = dt_("mr", [NCH, P, 512], BF16, kind="ExternalInput").ap()
    t["halo_ids"] = dt_("halo_ids", [P, 12], I32, kind="ExternalInput").ap()
    t["out"] = dt_("out", [FT, P, T_OWN], F32, kind="ExternalOutput").ap()

    with tile.TileContext(nc) as tc:
        _build_body(nc, tc, n_layers, t)
    nc.compile()
    return nc


def _build_body(nc, tc, n_layers, t):
    from contextlib import ExitStack
    with ExitStack() as ctx:
        persist = ctx.enter_context(tc.tile_pool(name="persist", bufs=1))
        # residual stream, transposed, with halos: x[ft] = [128, T_EXT] bf16
        x = [persist.tile([P, T_EXT], BF16, tag=f"x{ft}", name=f"x{ft}") for ft in range(FT)]
        ml_sb = [persist.tile([P, 512], BF16, tag=f"ml{c}", name=f"ml{c}") for c in range(NCH)]
        mr_sb = [persist.tile([P, 512], BF16, tag=f"mr{c}", name=f"mr{c}") for c in range(NCH)]
        for c in range(NCH):
            nc.sync.dma_start(ml_sb[c][:], t["ml"][c])
            nc.sync.dma_start(mr_sb[c][:], t["mr"][c])
        ones_f = persist.tile([P, 1], F32, tag="ones_f", name="ones_f")
        nc.vector.memset(ones_f[:], 1.0)
        ones_b = persist.tile([P, 1], BF16, tag="ones_b", name="ones_b")
        nc.scalar.activation(ones_b[:], ones_f[:], AF.Identity)
        from concourse.masks import make_identity
        ident_b = persist.tile([P, P], BF16, tag="identb", name="identb")
        make_identity(nc, ident_b[:])
        hid_sb = persist.tile([P, 12], I32, tag="hid", name="hid")
        nc.sync.dma_start(hid_sb[:], t["halo_ids"][:])
        eps_sb = persist.tile([P, 1], F32, tag="eps", name="eps")
        nc.vector.memset(eps_sb[:], EPS)

        # ---- embedding (+pos) -> transpose -> LN -> x^T ----
        with ExitStack() as ectx:
            esb = ectx.enter_context(tc.tile_pool(name="emb_sb", bufs=1))
            esb2 = ectx.enter_context(tc.tile_pool(name="emb_sb2", bufs=2))
            ids_sb = esb.tile([P, 12], I32, tag="ids", name="ids")
            nc.sync.dma_start(ids_sb[:], t["ids"][:])
            e = [esb.tile([P, HD], F32, tag=f"e{tt}", name=f"e{tt}") for tt in range(12)]
            p_sb = [esb.tile([P, HD], F32, tag=f"p{tt}", name=f"p{tt}") for tt in range(12)]
            ebf = [esb.tile([P, HD], BF16, tag=f"eb{tt}", name=f"eb{tt}") for tt in range(12)]
            for tt in range(12):
                nc.gpsimd.indirect_dma_start(
                    out=e[tt][:], out_offset=None, in_=t["emb"][:],
                    in_offset=bass.IndirectOffsetOnAxis(
                        ap=ids_sb[:, tt:tt + 1], axis=0))
                nc.sync.dma_start(p_sb[tt][:], t["pos"][tt * P:(tt + 1) * P, :])
            for tt in range(12):
                nc.vector.tensor_tensor(ebf[tt][:], e[tt][:], p_sb[tt][:],
                                        op=ALU.add)
            with tc.tile_pool(name="emb_ps", bufs=2, space="PSUM") as eps_ps:
                for ft in range(FT):
                    tr = eps_ps.tile([P, T_EXT], BF16, tag="tr", name="tr")
                    for tt in range(12):
                        nc.tensor.transpose(tr[:, tt * P:(tt + 1) * P],
                                            ebf[tt][:, ft * P:(ft + 1) * P],
                                            ident_b[:])
                    nc.vector.tensor_copy(x[ft][:], tr[:])
            eln_s = esb.tile([P, FT], F32, tag="elns", name="elns")
            nc.sync.dma_start(eln_s[:], t["eln_s"][:])
            eln_b = esb.tile([P, FT], F32, tag="elnb", name="elnb")
            nc.sync.dma_start(eln_b[:], t["eln_b"][:])
            with tc.tile_pool(name="emb_lnsb", bufs=1) as lsb_e, \
                 tc.tile_pool(name="emb_lnps", bufs=1, space="PSUM") as lps_e:
                _ln_transposed(nc, lsb_e, lps_e,
                               [x[ft][:] for ft in range(FT)],
                               [x[ft][:] for ft in range(FT)],
                               ones_b, eps_sb, eln_s, eln_b, T_EXT)

        # final-layer LN2 writes fp32 straight into xout (skips bf16 rounding
        # of the kernel output); allocated after the embed pools free
        xpool = ctx.enter_context(tc.tile_pool(name="xout", bufs=1))
        xout = [xpool.tile([P, T_OWN], F32, tag=f"xo{ft}", name=f"xo{ft}")
                for ft in range(FT)]
        for l in range(n_layers):
            _layer(nc, tc, t, l, x, ml_sb, mr_sb, ones_b, eps_sb, hid_sb,
                   exchange=(l < n_layers - 1), xout=xout,
                   final=(l == n_layers - 1))
        if n_layers == 0:
            for ft in range(FT):
                nc.vector.tensor_copy(xout[ft][:], x[ft][:, W:W + T_OWN])

        for ft in range(FT):
            nc.gpsimd.dma_start(t["out"][ft], xout[ft][:])


def _layer(nc, tc, t, l, x, ml_sb, mr_sb, ones_b, eps_sb, hid_sb, exchange,
           xout=None, final=False):
    from contextlib import ExitStack
    with ExitStack() as ctx:
        lsb = ctx.enter_context(tc.tile_pool(name=f"lsb{l}", bufs=1))

        def bias_tile(name, n=FT):
            bt = lsb.tile([P, n], F32, tag=f"b_{name}", name=f"b_{name}")
            nc.sync.dma_start(bt[:], t[name][l])
            return bt
        bq_sb = bias_tile("bq"); bk_sb = bias_tile("bk"); bo_sb = bias_tile("bo")
        b1_sb = bias_tile("b1", FFT); b2_sb = bias_tile("b2")
        ls1_sb = bias_tile("ls1"); lb1_sb = bias_tile("lb1")
        ls2_sb = bias_tile("ls2"); lb2_sb = bias_tile("lb2")

        qT = [lsb.tile([P, T_OWN], BF16, tag=f"qT{i}", name=f"qT{i}") for i in range(FT)]
        kT = [lsb.tile([P, T_EXT], BF16, tag=f"kT{i}", name=f"kT{i}") for i in range(FT)]
        v = [lsb.tile([P, NH, 65], BF16, tag=f"v{i}", name=f"v{i}") for i in range(12)]
        o = [lsb.tile([P, T_OWN], BF16, tag=f"o{i}", name=f"o{i}") for i in range(FT)]
        r2 = [lsb.tile([P, T_OWN], BF16, tag=f"r2_{i}", name=f"r2_{i}") for i in range(FT)]
        # aliases: qT tiles double as r1 (post-attention residual), kT's first
        # 1024 cols double as y (LN1 output) -- their producers are dead by then
        r1 = qT
        y = [kT[i][:, 0:T_OWN] for i in range(FT)]

        # ---- phases A+B interleaved: own-token work first, then the
        # attention chunks that need no halo, then halo work (so the halo
        # AllGather from the previous layer overlaps real compute) ----
        with tc.tile_pool(name=f"pa_w{l}", bufs=1) as wsb:
            wk_all = [wsb.tile([P, HD], BF16, tag=f"wk{kt}", name=f"wk{kt}")
                      for kt in range(FT)]
            wv_all = [wsb.tile([P, HD], BF16, tag=f"wv{kt}", name=f"wv{kt}")
                      for kt in range(FT)]
            for kt in range(FT):
                nc.sync.dma_start(wk_all[kt][:], t["wk"][l, kt])
                nc.sync.dma_start(wv_all[kt][:],
                                  t["wv"][l, kt * P:(kt + 1) * P, :])

            def q_half(wq_all, aps, mt, h2):
                ps = aps.tile([P, 512], F32, tag="pp", name="pp")
                for kt in range(FT):
                    nc.tensor.matmul(
                        ps[:], wq_all[mt][:, kt * P:(kt + 1) * P],
                        x[kt][:, W + h2 * 512:W + (h2 + 1) * 512],
                        start=(kt == 0), stop=(kt == FT - 1))
                nc.scalar.activation(qT[mt][:, h2 * 512:(h2 + 1) * 512],
                                     ps[:], AF.Identity,
                                     bias=bq_sb[:, mt:mt + 1])

            def k_chunk(aps, mt, c0, c1):
                ps = aps.tile([P, 512], F32, tag="pp", name="pp")
                for kt in range(FT):
                    nc.tensor.matmul(
                        ps[:, 0:c1 - c0], wk_all[mt][:, kt * P:(kt + 1) * P],
                        x[kt][:, c0:c1], start=(kt == 0), stop=(kt == FT - 1))
                nc.scalar.activation(kT[mt][:, c0:c1], ps[:, 0:c1 - c0],
                                     AF.Identity, bias=bk_sb[:, mt:mt + 1])

            def v_tt(aps, tt):
                for hf in range(2):
                    ps = aps.tile([P, 384], F32, tag="ppv", name="ppv", bufs=2)
                    for kt in range(FT):
                        nc.tensor.matmul(
                            ps[:], x[kt][:, tt * P:(tt + 1) * P],
                            wv_all[kt][:, hf * 384:(hf + 1) * 384],
                            start=(kt == 0), stop=(kt == FT - 1))
                    nc.scalar.activation(
                        v[tt][:, hf * 6:(hf + 1) * 6, 0:64],
                        ps[:].rearrange("p (h d) -> p h d", h=6), AF.Identity)
                nc.vector.memset(v[tt][:, :, 64:65], 1.0)

            def attn_chunk(bsb, bps, bps2, c):
                for h in range(NH):
                    ft, po = h // 2, (h % 2) * 64
                    sps = bps.tile([P, 6 * W], F32, tag="sps", name="sps")
                    for w in range(6):
                        nc.tensor.matmul(
                            sps[:, w * W:(w + 1) * W],
                            kT[ft][po:po + 64,
                                   c * W + w * P:c * W + (w + 1) * P],
                            qT[ft][po:po + 64, c * W:(c + 1) * W],
                            start=True, stop=True)
                    ex = bsb.tile([P, 6 * W], BF16, tag="ex", name="ex")
                    nc.scalar.activation(ex[:], sps[:], AF.Exp)
                    nc.vector.tensor_tensor(ex[:, 0:512], ex[:, 0:512],
                                            ml_sb[c][:], op=ALU.mult)
                    nc.vector.tensor_tensor(ex[:, 1024:1536], ex[:, 1024:1536],
                                            mr_sb[c][:], op=ALU.mult)
                    ops = bps2.tile([P, W], F32, tag="ops", name="ops")
                    for w in range(6):
                        nc.tensor.matmul(
                            ops[0:65, :],
                            v[c * 2 + w][:, h, :],
                            ex[:, w * W:(w + 1) * W],
                            start=(w == 0), stop=(w == 5))
                    rs0 = bsb.tile([1, W], F32, tag="rs0", name="rs0")
                    nc.vector.tensor_copy(rs0[:], ops[64:65, :])
                    rs = bsb.tile([1, W], F32, tag="rs", name="rs")
                    nc.vector.reciprocal_approx_fast(rs[:], rs0[:])
                    rb = bsb.tile([64, W], F32, tag="rb", name="rb")
                    nc.gpsimd.partition_broadcast(rb[:], rs[:], channels=64)
                    nc.vector.tensor_tensor(
                        o[ft][po:po + 64, c * W:(c + 1) * W],
                        ops[0:64, :], rb[:], op=ALU.mult)

            # region 1: own-token QKV (h2=0-dependent work first so the
            # previous layer's LN2 h2=1 tail overlaps)
            with tc.tile_pool(name=f"pa_sb{l}", bufs=1) as asb, \
                 tc.tile_pool(name=f"pa_ps{l}", bufs=4, space="PSUM") as aps:
                wq_all = [asb.tile([P, HD], BF16, tag=f"wq{mt}",
                                   name=f"wq{mt}") for mt in range(FT)]
                for mt in range(FT):
                    nc.sync.dma_start(wq_all[mt][:], t["wq"][l, mt])
                for mt in range(FT):
                    q_half(wq_all, aps, mt, 0)
                for mt in range(FT):
                    k_chunk(aps, mt, W, W + 512)
                for tt in range(2, 6):
                    v_tt(aps, tt)
                for mt in range(FT):
                    q_half(wq_all, aps, mt, 1)
                for mt in range(FT):
                    k_chunk(aps, mt, W + 512, W + 1024)
                for tt in range(6, 10):
                    v_tt(aps, tt)
            # region 2: attention chunks that only need own tokens
            with tc.tile_pool(name=f"pb_sb{l}a", bufs=3) as bsb, \
                 tc.tile_pool(name=f"pb_ps{l}a", bufs=2, space="PSUM") as bps, \
                 tc.tile_pool(name=f"pb_ps2{l}a", bufs=2,
                              space="PSUM") as bps2:
                attn_chunk(bsb, bps, bps2, 1)
                attn_chunk(bsb, bps, bps2, 2)
            # region 3: halo-dependent K/V (waits on the AllGather)
            with tc.tile_pool(name=f"pa_ps{l}h", bufs=4, space="PSUM") as aps:
                for mt in range(FT):
                    k_chunk(aps, mt, 0, W)
                    k_chunk(aps, mt, W + 1024, T_EXT)
                for tt in (0, 1, 10, 11):
                    v_tt(aps, tt)
            # region 4: halo-dependent attention chunks
            with tc.tile_pool(name=f"pb_sb{l}b", bufs=3) as bsb, \
                 tc.tile_pool(name=f"pb_ps{l}b", bufs=2, space="PSUM") as bps, \
                 tc.tile_pool(name=f"pb_ps2{l}b", bufs=2,
                              space="PSUM") as bps2:
                attn_chunk(bsb, bps, bps2, 0)
                attn_chunk(bsb, bps, bps2, 3)

        # ---- phase C: O-proj + residual (r1 <- x + O@Wo + bo), h2-major ----
        with tc.tile_pool(name=f"pc_sb{l}", bufs=1) as csb, \
             tc.tile_pool(name=f"pc_ps{l}", bufs=4, space="PSUM") as cps:
            wo_all = [csb.tile([P, HD], BF16, tag=f"wo{mt}", name=f"wo{mt}")
                      for mt in range(FT)]
            for mt in range(FT):
                nc.sync.dma_start(wo_all[mt][:], t["wo"][l, mt])
            for h2 in range(2):
                for mt in range(FT):
                    ps = cps.tile([P, 512], F32, tag="ppo", name="ppo")
                    for kt in range(FT):
                        nc.tensor.matmul(
                            ps[:], wo_all[mt][:, kt * P:(kt + 1) * P],
                            o[kt][:, h2 * 512:(h2 + 1) * 512],
                            start=(kt == 0), stop=(kt == FT - 1))
                    nc.vector.scalar_tensor_tensor(
                        out=r1[mt][:, h2 * 512:(h2 + 1) * 512], in0=ps[:],
                        scalar=bo_sb[:, mt:mt + 1],
                        in1=x[mt][:, W + h2 * 512:W + (h2 + 1) * 512],
                        op0=ALU.add, op1=ALU.add)
                # LN1 for this half: y <- LN(r1)*s+b
                cs = slice(h2 * 512, (h2 + 1) * 512)
                with tc.tile_pool(name=f"ln1_sb{l}_{h2}", bufs=2) as l1sb, \
                     tc.tile_pool(name=f"ln1_ps{l}_{h2}", bufs=1,
                                  space="PSUM") as l1ps:
                    _ln_transposed(nc, l1sb, l1ps,
                                   [r1[ft][:, cs] for ft in range(FT)],
                                   [y[ft][:, cs] for ft in range(FT)],
                                   ones_b, eps_sb, ls1_sb, lb1_sb, 512)

        # ---- phase D: FFN + residual + LN2 -> x middle (per token-half);
        # halo edge blocks are DMA'd to DRAM as soon as their LN2 half lands
        edram = ctx.enter_context(
            tc.tile_pool(name=f"pe_dram{l}", bufs=1, space="DRAM")) \
            if exchange else None
        if exchange:
            b_in = edram.tile([2, FT, P, W], BF16, tag="bin", name="bin")
            b_out = edram.tile([4 * 2 * FT * P, W], BF16, tag="bout",
                               name="bout")
        for h2 in range(2):
            cs = slice(h2 * 512, (h2 + 1) * 512)
            csx = slice(W + h2 * 512, W + (h2 + 1) * 512)
            with ExitStack() as dctx:
                dsb = dctx.enter_context(
                    tc.tile_pool(name=f"pd_sb{l}_{h2}", bufs=3))
                zps = dctx.enter_context(
                    tc.tile_pool(name=f"pd_psz{l}_{h2}", bufs=1, space="PSUM"))
                fps = dctx.enter_context(
                    tc.tile_pool(name=f"pd_psf{l}_{h2}", bufs=2, space="PSUM"))
                zp = [zps.tile([P, 512], F32, tag=f"z{mt}", name=f"z{mt}") for mt in range(FT)]
                for ms in range(FFT):
                    w1_sb = dsb.tile([P, HD], BF16, tag="w1s", name="w1s")
                    nc.sync.dma_start(w1_sb[:], t["w1"][l, ms])
                    fp = fps.tile([P, 512], F32, tag="fp", name="fp")
                    for kt in range(FT):
                        nc.tensor.matmul(fp[:], w1_sb[:, kt * P:(kt + 1) * P],
                                         y[kt][:, cs], start=(kt == 0),
                                         stop=(kt == FT - 1))
                    f_sb = dsb.tile([P, 512], BF16, tag="fsb", name="fsb")
                    nc.scalar.activation(f_sb[:], fp[:], AF.Gelu,
                                         bias=b1_sb[:, ms:ms + 1])
                    w2_sb = dsb.tile([P, HD], BF16, tag="w2s", name="w2s")
                    nc.sync.dma_start(w2_sb[:],
                                      t["w2"][l, ms * P:(ms + 1) * P, :])
                    for mt in range(FT):
                        nc.tensor.matmul(zp[mt][:],
                                         w2_sb[:, mt * P:(mt + 1) * P],
                                         f_sb[:], start=(ms == 0),
                                         stop=(ms == FFT - 1))
                for mt in range(FT):
                    nc.vector.scalar_tensor_tensor(
                        out=r2[mt][:, cs], in0=zp[mt][:],
                        scalar=b2_sb[:, mt:mt + 1], in1=y[mt][:, cs],
                        op0=ALU.add, op1=ALU.add)
            ln2_out = ([xout[ft][:, cs] for ft in range(FT)] if final
                       else [x[ft][:, csx] for ft in range(FT)])
            with tc.tile_pool(name=f"ln2_sb{l}_{h2}", bufs=2) as l2sb, \
                 tc.tile_pool(name=f"ln2_ps{l}_{h2}", bufs=1,
                              space="PSUM") as l2ps:
                _ln_transposed(nc, l2sb, l2ps,
                               [r2[ft][:, cs] for ft in range(FT)],
                               ln2_out,
                               ones_b, eps_sb, ls2_sb, lb2_sb, 512)
            if exchange:
                for ft in range(FT):
                    if h2 == 0:
                        nc.sync.dma_start(b_in[0, ft], x[ft][:, W:2 * W])
                    else:
                        nc.sync.dma_start(b_in[1, ft],
                                          x[ft][:, T_OWN:T_OWN + W])

        # ---- phase E: halo exchange ----
        if exchange:
            nc.gpsimd.collective_compute(
                "AllGather", ALU.bypass,
                replica_groups=[[0, 1, 2, 3], [4, 5, 6, 7]],
                ins=[b_in[:].opt()], outs=[b_out[:].opt()])
            for side in range(2):
                for ft in range(FT):
                    dst = (x[ft][:, 0:W] if side == 0
                           else x[ft][:, T_OWN + W:T_EXT])
                    nc.gpsimd.indirect_dma_start(
                        out=dst, out_offset=None, in_=b_out[:],
                        in_offset=bass.IndirectOffsetOnAxis(
                            ap=hid_sb[:, side * FT + ft:side * FT + ft + 1],
                            axis=0))


# ---------------- host side ----------------

def _blocked(w, n_k, n_m):
    """[n_k*128, n_m*128] -> [n_m, 128, n_k, 128] (lhsT strips by out-tile)."""
    return np.ascontiguousarray(
        w.reshape(n_k, P, n_m, P).transpose(2, 1, 0, 3))


def _bias_lay(b, n):
    return np.ascontiguousarray(b.reshape(n, P).T)


def prepare(inputs):
    """Build per-core in_maps from full inputs."""
    bf = ml_dtypes.bfloat16
    ids_full = np.asarray(inputs["input_ids"]).astype(np.int32)
    am = np.asarray(inputs["attention_mask"]).astype(np.int32)
    emb_word = np.asarray(inputs["emb_word"], dtype=np.float32)
    emb_pos = np.asarray(inputs["emb_pos"], dtype=np.float32)
    Wq = np.asarray(inputs["Wq"], np.float32) / np.sqrt(DH)
    bq = np.asarray(inputs["bq"], np.float32) / np.sqrt(DH)
    Wk = np.asarray(inputs["Wk"], np.float32)
    bk = np.asarray(inputs["bk"], np.float32)
    Wv = np.asarray(inputs["Wv"], np.float32)
    bv = np.asarray(inputs["bv"], np.float32)
    Wo = np.asarray(inputs["Wo"], np.float32)
    bo = np.asarray(inputs["bo"], np.float32)
    W1 = np.asarray(inputs["W1"], np.float32)
    b1 = np.asarray(inputs["b1"], np.float32)
    W2 = np.asarray(inputs["W2"], np.float32)
    b2 = np.asarray(inputs["b2"], np.float32)
    assert np.all(am == 1), "general attention_mask needs mid-tile masks too"

    shared = {
        "emb_word": emb_word,
        "eln_s": _bias_lay(np.asarray(inputs["emb_ln_s"], np.float32), FT),
        "eln_b": _bias_lay(np.asarray(inputs["emb_ln_b"], np.float32), FT),
        "wq": np.stack([_blocked(Wq[i], FT, FT) for i in range(L)]).astype(bf),
        "wk": np.stack([_blocked(Wk[i], FT, FT) for i in range(L)]).astype(bf),
        "wv": Wv.astype(bf),
        "wo": np.stack([_blocked(Wo[i], FT, FT) for i in range(L)]).astype(bf),
        "w1": np.stack([_blocked(W1[i], FT, FFT) for i in range(L)]).astype(bf),
        "w2": W2.astype(bf),
        "bq": np.stack([_bias_lay(bq[i], FT) for i in range(L)]),
        "bk": np.stack([_bias_lay(bk[i], FT) for i in range(L)]),
        "bo": np.stack([_bias_lay(bv[i] @ Wo[i] + bo[i], FT)
                        for i in range(L)]),
        "b1": np.stack([_bias_lay(b1[i], FFT) for i in range(L)]),
        "b2": np.stack([_bias_lay(b2[i], FT) for i in range(L)]),
        "ls1": np.stack([_bias_lay(np.asarray(inputs["ln1_s"], np.float32)[i],
                                   FT) for i in range(L)]),
        "lb1": np.stack([_bias_lay(np.asarray(inputs["ln1_b"], np.float32)[i],
                                   FT) for i in range(L)]),
        "ls2": np.stack([_bias_lay(np.asarray(inputs["ln2_s"], np.float32)[i],
                                   FT) for i in range(L)]),
        "lb2": np.stack([_bias_lay(np.asarray(inputs["ln2_b"], np.float32)[i],
                                   FT) for i in range(L)]),
    }

    in_maps = []
    i_idx = np.arange(W)
    for core in range(N_CORES):
        b, sb = core // 4, core % 4
        s0 = sb * T_OWN
        ext_pos = np.clip(np.arange(s0 - W, s0 + T_OWN + W), 0, S - 1)
        m = dict(shared)
        m["ids"] = np.ascontiguousarray(
            ids_full[b, ext_pos].reshape(12, P).T)
        m["pos"] = np.ascontiguousarray(emb_pos[ext_pos])
        # masks: global chunk gc, window key j in [0,768), query i in [0,256):
        #   key_abs = gc*W - W + j ; allowed = |j - W - i| <= W
        #             & 0 <= key_abs < S & attention_mask[b, key_abs]
        mlm = np.zeros((NCH, P, 512), np.float32)
        mrm = np.zeros((NCH, P, 512), np.float32)
        for c in range(NCH):
            gc = sb * NCH + c
            for kt2 in range(2):
                for mm_, j0 in ((mlm, 0), (mrm, 512)):
                    j = j0 + kt2 * P + np.arange(P)[:, None]
                    key_abs = gc * W - W + j
                    ok = (np.abs(j - W - i_idx[None, :]) <= W)
                    ok &= (key_abs >= 0) & (key_abs < S)
                    ok &= am[b, np.clip(key_abs, 0, S - 1)] > 0
                    mm_[c, :, kt2 * W:(kt2 + 1) * W] = ok
        m["ml"] = mlm.astype(bf)
        m["mr"] = mrm.astype(bf)
        # halo row ids into the gathered [4, 2, FT, 128, W] row table
        hid = np.zeros((2, FT, P), np.int64)
        for side in range(2):
            nb = sb - 1 if side == 0 else sb + 1
            if 0 <= nb <= 3:
                osd = 1 - side  # left halo <- neighbor's right block
                for ft in range(FT):
                    hid[side, ft] = ((nb * 2 + osd) * FT + ft) * P \
                        + np.arange(P)
            else:
                for ft in range(FT):
                    hid[side, ft] = ((sb * 2 + side) * FT + ft) * P \
                        + np.arange(P)
        m["halo_ids"] = np.ascontiguousarray(
            hid.reshape(12, P).T.astype(np.int32))
        in_maps.append(m)
    return in_maps


_NC_CACHE = {}


def get_nc(n_layers=L):
    if n_layers not in _NC_CACHE:
        _NC_CACHE[n_layers] = build_nc(n_layers)
    return _NC_CACHE[n_layers]


def run(inputs, n_layers=L, trace=False):
    nc = get_nc(n_layers)
    in_maps = prepare(inputs)
    res = bass_utils.run_bass_kernel_spmd(
        nc, in_maps, core_ids=list(range(N_CORES)), trace=trace)
    outs = np.empty((B, S, HD), np.float32)
    for core in range(N_CORES):
        b, sb = core // 4, core % 4
        ot = res.results[core]["out"]  # [FT, 128, T_OWN] f32
        outs[b, sb * T_OWN:(sb + 1) * T_OWN] = \
            np.asarray(ot, dtype=np.float32).reshape(HD, T_OWN).T
    return outs, res


def kernel(**inputs) -> np.ndarray:
    out, _ = run(inputs)
    return out
